# revision 6
# baseline (speedup 1.0000x reference)
"""BiMamba (bidirectional Mamba block) on 8 TRN2 NeuronCores.

Sharding: 4 independent (batch, direction) units x 2-way split of
d_inner (2048 -> 2x1024). Core c = (b=c//4, dir=(c//2)%2, half=c%2).
All cores run ONE SPMD program; per-core differences are folded into the
host-prepared inputs (x transposed/reversed, weights sliced and channel-
permuted so the core's own d_inner half is always channels 0..1023).
Each core computes a full-depth partial of out[b] over its half; the
host sums partials, un-reverses the reverse direction, adds directions.

Per-core pipeline:
  A: in_proj (PE fp16): xi full 2048ch (x_dbl needs all) + z half;
     causal dw-conv = 4 accumulating diagonal matmuls (PE); Silu (ACT)
  B: x_dbl = Wx @ xc (PE); dt = softplus via Exp(.+bdt)+Ln(1+.) (ACT);
     B/C rows replicated to 128 partitions via 0-stride DMA
  C (per d-tile pair, per n): dA = Exp(dt*A[d,n]) (ACT per-partition
     scale); dBu = (dt*u) o B_n (DVE TT fp16 2x); h = tensor_tensor_scan
     (DVE); G = h o C_n (DVE); y = sum_n G_n via identity-matmul PSUM
     accumulation (PE); gating y2 = (y + u*Dskip)*silu(z) (DVE)
  D: out_proj partial (PE) -> DRAM fp32
"""
import os
import sys
import types

sys.path.insert(0, "/opt/trn_rl_repo")

import numpy as np

# ---- NTFF profile hook shim (trace path only; harmless otherwise) ----
if "antenv.axon_hooks" not in sys.modules:
    _m = types.ModuleType("antenv.axon_hooks")
    _m._HOOK = None
    _m.set_axon_ntff_profile_hook = lambda h, _m=_m: setattr(_m, "_HOOK", h)
    _m.get_axon_ntff_profile_hook = lambda _m=_m: _m._HOOK
    sys.modules["antenv.axon_hooks"] = _m

import concourse.bacc as bacc
import concourse.tile as tile
from concourse import mybir
from concourse.bass_utils import run_bass_kernel_spmd

f32 = mybir.dt.float32
f16 = mybir.dt.float16

DT_RANK = 64
N_STATE = 16
K_CONV = 4
P = 128


def build(L=1024, DM=1024, DH=1024):
    MULT = mybir.AluOpType.mult
    ADD = mybir.AluOpType.add
    ACT = mybir.ActivationFunctionType

    nc = bacc.Bacc("TRN2")
    DI = 2 * DH                      # full d_inner
    KT = DM // P                     # k-tiles over d_model (8)
    XT = DI // P                     # xi tiles (16)
    ZT = DH // P                     # z / scan tiles (8)
    FD = 512                         # matmul free-dim (one PSUM bank fp32)
    NF = L // FD
    NX = DT_RANK + 2 * N_STATE       # 96

    xT = nc.dram_tensor("xT", [DM, L], f16, kind="ExternalInput")
    winT = nc.dram_tensor("winT", [P, (DI + DH) // P, KT, P], f16, kind="ExternalInput")
    convd = nc.dram_tensor("convd", [P, XT, K_CONV, P], f16, kind="ExternalInput")
    bconv = nc.dram_tensor("bconv", [P, XT], f32, kind="ExternalInput")
    wxT = nc.dram_tensor("wxT", [DI, NX], f16, kind="ExternalInput")
    wdtT = nc.dram_tensor("wdtT", [DT_RANK, DH], f16, kind="ExternalInput")
    bdt = nc.dram_tensor("bdt", [P, ZT], f32, kind="ExternalInput")
    At = nc.dram_tensor("At", [P, ZT * N_STATE], f32, kind="ExternalInput")
    dskip = nc.dram_tensor("dskip", [P, ZT], f32, kind="ExternalInput")
    woutT = nc.dram_tensor("woutT", [P, KT, ZT, P], f16, kind="ExternalInput")
    out = nc.dram_tensor("out", [DM, L], f32, kind="ExternalOutput")

    bcscr = nc.dram_tensor("bcscr", [2 * N_STATE, L], f16)   # internal
    ident_dr = nc.inline_tensor(np.eye(P, dtype=np.float16), "ident")

    with tile.TileContext(nc) as tc:
        with tc.tile_pool(name="res", bufs=1) as res, \
             tc.tile_pool(name="wpool", bufs=4) as wpool, \
             tc.tile_pool(name="bcp", bufs=3) as bcp, \
             tc.tile_pool(name="wk", bufs=2) as wk, \
             tc.tile_pool(name="scw", bufs=2) as scw, \
             tc.tile_pool(name="ps", bufs=4, space="PSUM") as ps:

            # ---- resident tiles ----
            xT_sb = res.tile([P, KT, L], f16)       # x^T, k-tile major
            xi = res.tile([P, XT, 3 + L], f16)      # pre-conv xi (3 halo cols)
            xc = res.tile([P, XT, L], f16)          # silu(conv(xi)) = u
            sz = res.tile([P, ZT, L], f16)          # silu(z)
            dt = res.tile([P, ZT, L], f16)          # softplus dt
            y2 = res.tile([P, ZT, L], f16)          # gated scan output
            xdbl = res.tile([P, L], f16)            # x_dbl rows (96 used)
            ident = res.tile([P, P], f16)
            At_sb = res.tile([P, ZT * N_STATE], f32)
            bdt_sb = res.tile([P, ZT], f32)
            dsk_sb = res.tile([P, ZT], f32)
            bcv_sb = res.tile([P, XT], f32)
            wdt_sb = res.tile([DT_RANK, DH], f16)

            nc.sync.dma_start(ident[:], ident_dr[:])
            nc.sync.dma_start(At_sb[:], At[:])
            nc.sync.dma_start(bdt_sb[:], bdt[:])
            nc.sync.dma_start(dsk_sb[:], dskip[:])
            nc.sync.dma_start(bcv_sb[:], bconv[:])
            nc.sync.dma_start(wdt_sb[:], wdtT[:])
            for k in range(KT):
                nc.sync.dma_start(xT_sb[:, k, :], xT[k * P:(k + 1) * P, :])
            for i in range(XT):
                nc.gpsimd.memset(xi[:, i, 0:3], 0.0)

            # ---- Phase A+B interleaved: per xi tile do in_proj -> conv
            # -> x_dbl accumulation, so dt (the scan gate) is ready as
            # early as possible; z tiles (only needed at gating) go last.
            pxd = ps.tile([P, L], f32, tag="mm")
            for i in range(XT):
                pacc = ps.tile([P, L], f32, tag="mm")
                wcol = wpool.tile([P, KT, P], f16, tag="wcol")
                nc.sync.dma_start(wcol[:], winT[:, i, :, :])
                for k in range(KT):
                    for f in range(NF):
                        nc.tensor.matmul(
                            pacc[:, f * FD:(f + 1) * FD], wcol[:, k, :],
                            xT_sb[:, k, f * FD:(f + 1) * FD],
                            start=(k == 0), stop=(k == KT - 1))
                nc.scalar.copy(xi[:, i, 3:3 + L], pacc[:])

                pcv = ps.tile([P, L], f32, tag="mm")
                cdall = wpool.tile([P, K_CONV, P], f16, tag="cd")
                nc.sync.dma_start(cdall[:], convd[:, i, :, :])
                for j in range(K_CONV):
                    for f in range(NF):
                        nc.tensor.matmul(
                            pcv[:, f * FD:(f + 1) * FD], cdall[:, j, :],
                            xi[:, i, j + f * FD: j + (f + 1) * FD],
                            start=(j == 0), stop=(j == K_CONV - 1))
                nc.scalar.activation(xc[:, i, :], pcv[:], ACT.Silu,
                                     bias=bcv_sb[:, i:i + 1])

                wchunk = wpool.tile([P, NX], f16, tag="wx")
                nc.sync.dma_start(wchunk[:], wxT[i * P:(i + 1) * P, :])
                for f in range(NF):
                    nc.tensor.matmul(
                        pxd[:NX, f * FD:(f + 1) * FD], wchunk[:],
                        xc[:, i, f * FD:(f + 1) * FD],
                        start=(i == 0), stop=(i == XT - 1))
            nc.scalar.copy(xdbl[:NX, :], pxd[:NX, :])
            nc.sync.dma_start(bcscr[:], xdbl[DT_RANK:DT_RANK + 2 * N_STATE, :])

            for d in range(ZT):
                pdt = ps.tile([P, L], f32, tag="mm")
                for f in range(NF):
                    nc.tensor.matmul(
                        pdt[:, f * FD:(f + 1) * FD],
                        wdt_sb[:, d * P:(d + 1) * P],
                        xdbl[:DT_RANK, f * FD:(f + 1) * FD],
                        start=True, stop=True)
                tmp = wk.tile([P, L], f32, tag="f32tmp")
                nc.scalar.activation(tmp[:], pdt[:], ACT.Exp,
                                     bias=bdt_sb[:, d:d + 1])
                nc.scalar.activation(dt[:, d, :], tmp[:], ACT.Ln, bias=1.0)

            # z projection tiles (feed gating, needed ~60us into phase C)
            for zi in range(ZT):
                pacc = ps.tile([P, L], f32, tag="mm")
                wcol = wpool.tile([P, KT, P], f16, tag="wcol")
                nc.sync.dma_start(wcol[:], winT[:, XT + zi, :, :])
                for k in range(KT):
                    for f in range(NF):
                        nc.tensor.matmul(
                            pacc[:, f * FD:(f + 1) * FD], wcol[:, k, :],
                            xT_sb[:, k, f * FD:(f + 1) * FD],
                            start=(k == 0), stop=(k == KT - 1))
                nc.scalar.activation(sz[:, zi, :], pacc[:], ACT.Silu)

            # ---- Phase C: selective scan, d-tile pairs ----
            for dp in range(ZT // 2):
                ds = (2 * dp, 2 * dp + 1)
                yps = {}
                dus = {}
                for d in ds:
                    ypt = ps.tile([P, L], f32, tag="mm")
                    yps[d] = ypt
                    du = wk.tile([P, L], f16, tag="du")
                    nc.vector.tensor_tensor(du[:], dt[:, d, :], xc[:, d, :],
                                            MULT)
                    dus[d] = du
                for n in range(N_STATE):
                    Bn = bcp.tile([P, L], f16, tag="Bn")
                    Cn = bcp.tile([P, L], f16, tag="Cn")
                    nc.sync.dma_start(Bn[:], bcscr[n, :].partition_broadcast(P))
                    nc.sync.dma_start(
                        Cn[:], bcscr[N_STATE + n, :].partition_broadcast(P))
                    for d in ds:
                        dA = scw.tile([P, L], f16, tag="dA")
                        nc.scalar.activation(
                            dA[:], dt[:, d, :], ACT.Exp,
                            scale=At_sb[:, d * N_STATE + n:d * N_STATE + n + 1])
                        dBu = scw.tile([P, L], f16, tag="dBu")
                        nc.vector.tensor_tensor(dBu[:], dus[d][:], Bn[:], MULT)
                        H = scw.tile([P, L], f16, tag="H")
                        nc.vector.tensor_tensor_scan(H[:], dA[:], dBu[:], 0.0,
                                                     MULT, ADD)
                        G = scw.tile([P, L], f16, tag="G")
                        nc.vector.tensor_tensor(G[:], H[:], Cn[:], MULT)
                        for f in range(NF):
                            nc.tensor.matmul(
                                yps[d][:, f * FD:(f + 1) * FD], ident[:],
                                G[:, f * FD:(f + 1) * FD],
                                start=(n == 0), stop=(n == N_STATE - 1))
                for d in ds:
                    y1 = wk.tile([P, L], f32, tag="f32tmp")
                    nc.vector.scalar_tensor_tensor(
                        y1[:], xc[:, d, :], dsk_sb[:, d:d + 1], yps[d][:],
                        MULT, ADD)
                    nc.vector.tensor_tensor(y2[:, d, :], y1[:], sz[:, d, :],
                                            MULT)

            # ---- Phase D: out_proj partial ----
            for m in range(KT):
                po = ps.tile([P, L], f32, tag="mm")
                wcol = wpool.tile([P, ZT, P], f16, tag="wcol")
                nc.sync.dma_start(wcol[:], woutT[:, m, :, :])
                for k in range(ZT):
                    for f in range(NF):
                        nc.tensor.matmul(
                            po[:, f * FD:(f + 1) * FD], wcol[:, k, :],
                            y2[:, k, f * FD:(f + 1) * FD],
                            start=(k == 0), stop=(k == ZT - 1))
                osb = wk.tile([P, L], f32, tag="f32tmp")
                nc.scalar.copy(osb[:], po[:])
                nc.sync.dma_start(out[m * P:(m + 1) * P, :], osb[:])

    nc.compile()
    return nc


def _prep_core(inputs, b, rev, half, L=1024, DM=1024, DH=1024):
    """Host-side slicing/permutation for one core's in_map.

    Channel permutation puts the core's own d_inner half at channels
    0..DH-1 so the SPMD program can use fixed tile indices for u/scan.
    """
    sfx = "r" if rev else "f"
    DI = 2 * DH
    x = np.asarray(inputs["x"])[b].astype(np.float32)     # [L, DM]
    if rev:
        x = x[::-1]
    Win = np.asarray(inputs[f"Win_{sfx}"])
    Wconv = np.asarray(inputs[f"Wconv_{sfx}"])
    bconv = np.asarray(inputs[f"bconv_{sfx}"])
    Wx = np.asarray(inputs[f"Wx_{sfx}"])
    Wdt = np.asarray(inputs[f"Wdt_{sfx}"])
    bdt = np.asarray(inputs[f"bdt_{sfx}"])
    Alog = np.asarray(inputs[f"Alog_{sfx}"])
    Dskip = np.asarray(inputs[f"Dskip_{sfx}"])
    Wout = np.asarray(inputs[f"Wout_{sfx}"])

    own = np.arange(half * DH, (half + 1) * DH)
    oth = np.arange((1 - half) * DH, (2 - half) * DH)
    perm = np.concatenate([own, oth])                     # xi channel order
    XT, ZT = DI // P, DH // P

    winT = np.concatenate(
        [Win[:DI][perm].T, Win[DI + half * DH:DI + (half + 1) * DH].T], axis=1)
    ET = (DI + DH) // P
    KT = DM // P
    winT = winT.reshape(KT, P, ET, P).transpose(1, 2, 0, 3)  # [p, e, k, c]
    Wcp = Wconv[perm].astype(np.float16)
    convd = np.zeros((P, XT, K_CONV, P), np.float16)
    pi = np.arange(P)
    for i in range(XT):
        for j in range(K_CONV):
            convd[pi, i, j, pi] = Wcp[i * P + pi, j]
    A = -np.exp(Alog[own])                                # [DH, 16]
    return {
        "xT": np.ascontiguousarray(x.T).astype(np.float16),
        "winT": np.ascontiguousarray(winT).astype(np.float16),
        "convd": convd,
        "bconv": np.ascontiguousarray(
            bconv[perm].reshape(XT, P).T).astype(np.float32),
        "wxT": np.ascontiguousarray(Wx[:, perm].T).astype(np.float16),
        "wdtT": np.ascontiguousarray(Wdt[own].T).astype(np.float16),
        "bdt": np.ascontiguousarray(
            bdt[own].reshape(ZT, P).T).astype(np.float32),
        "At": np.ascontiguousarray(
            A.reshape(ZT, P, N_STATE).transpose(1, 0, 2).reshape(
                P, ZT * N_STATE)).astype(np.float32),
        "dskip": np.ascontiguousarray(
            Dskip[own].reshape(ZT, P).T).astype(np.float32),
        "woutT": np.ascontiguousarray(Wout[:, own].T.reshape(DH // P, P, DM // P, P).transpose(1, 2, 0, 3)).astype(np.float16),
    }


_NC_CACHE = {}


def kernel(**inputs) -> np.ndarray:
    L, DM = 1024, 1024
    if "nc" not in _NC_CACHE:
        _NC_CACHE["nc"] = build(L=L, DM=DM, DH=1024)
    nc = _NC_CACHE["nc"]

    in_maps = [
        _prep_core(inputs, c // 4, bool((c // 2) % 2), c % 2)
        for c in range(8)
    ]

    import jax
    jax.devices()
    trace = os.environ.get("BIMAMBA_TRACE") == "1"
    if trace:
        from trn_agent_boot.trn_boot import _ntff_profile_via_ctypes
        import antenv.axon_hooks as ah
        if ah.get_axon_ntff_profile_hook() is None:
            ah.set_axon_ntff_profile_hook(
                _ntff_profile_via_ctypes("/opt/axon/libaxon_pjrt.so"))
    res = run_bass_kernel_spmd(nc, in_maps, list(range(8)), trace=trace)
    _NC_CACHE["exec_time_ns"] = res.exec_time_ns

    B = np.asarray(inputs["x"]).shape[0]
    outp = np.zeros((B, L, DM), np.float32)
    for c in range(8):
        b, rev = c // 4, (c // 2) % 2
        part = np.asarray(res.results[c]["out"]).T        # [L, DM]
        if rev:
            part = part[::-1]
        outp[b] += part
    return outp


# revision 7
# speedup vs baseline: 1.0107x; 1.0107x over previous
"""BiMamba (bidirectional Mamba block) on 8 TRN2 NeuronCores.

Sharding: 4 independent (batch, direction) units x 2-way split of
d_inner (2048 -> 2x1024). Core c = (b=c//4, dir=(c//2)%2, half=c%2).
All cores run ONE SPMD program; per-core differences are folded into the
host-prepared inputs (x transposed/reversed, weights sliced and channel-
permuted so the core's own d_inner half is always channels 0..1023).
Each core computes a full-depth partial of out[b] over its half; the
host sums partials, un-reverses the reverse direction, adds directions.

Per-core pipeline:
  A: in_proj (PE fp16): xi full 2048ch (x_dbl needs all) + z half;
     causal dw-conv = 4 accumulating diagonal matmuls (PE); Silu (ACT)
  B: x_dbl = Wx @ xc (PE); dt = softplus via Exp(.+bdt)+Ln(1+.) (ACT);
     B/C rows replicated to 128 partitions via 0-stride DMA
  C (per d-tile pair, per n): dA = Exp(dt*A[d,n]) (ACT per-partition
     scale); dBu = (dt*u) o B_n (DVE TT fp16 2x); h = tensor_tensor_scan
     (DVE); G = h o C_n (DVE); y = sum_n G_n via identity-matmul PSUM
     accumulation (PE); gating y2 = (y + u*Dskip)*silu(z) (DVE)
  D: out_proj partial (PE) -> DRAM fp32
"""
import os
import sys
import types

sys.path.insert(0, "/opt/trn_rl_repo")

import numpy as np

# ---- NTFF profile hook shim (trace path only; harmless otherwise) ----
if "antenv.axon_hooks" not in sys.modules:
    _m = types.ModuleType("antenv.axon_hooks")
    _m._HOOK = None
    _m.set_axon_ntff_profile_hook = lambda h, _m=_m: setattr(_m, "_HOOK", h)
    _m.get_axon_ntff_profile_hook = lambda _m=_m: _m._HOOK
    sys.modules["antenv.axon_hooks"] = _m

import concourse.bacc as bacc
import concourse.tile as tile
from concourse import mybir
from concourse.bass_utils import run_bass_kernel_spmd

f32 = mybir.dt.float32
f16 = mybir.dt.float16

DT_RANK = 64
N_STATE = 16
K_CONV = 4
P = 128


def build(L=1024, DM=1024, DH=1024):
    MULT = mybir.AluOpType.mult
    ADD = mybir.AluOpType.add
    ACT = mybir.ActivationFunctionType

    nc = bacc.Bacc("TRN2")
    DI = 2 * DH                      # full d_inner
    KT = DM // P                     # k-tiles over d_model (8)
    XT = DI // P                     # xi tiles (16)
    ZT = DH // P                     # z / scan tiles (8)
    FD = 512                         # matmul free-dim (one PSUM bank fp32)
    NF = L // FD
    NX = DT_RANK + 2 * N_STATE       # 96

    xT = nc.dram_tensor("xT", [DM, L], f16, kind="ExternalInput")
    winT = nc.dram_tensor("winT", [P, (DI + DH) // P, KT, P], f16, kind="ExternalInput")
    convd = nc.dram_tensor("convd", [P, XT, K_CONV, P], f16, kind="ExternalInput")
    bconv = nc.dram_tensor("bconv", [P, XT], f32, kind="ExternalInput")
    wxT = nc.dram_tensor("wxT", [DI, NX], f16, kind="ExternalInput")
    wdtT = nc.dram_tensor("wdtT", [DT_RANK, DH], f16, kind="ExternalInput")
    bdt = nc.dram_tensor("bdt", [P, ZT], f32, kind="ExternalInput")
    At = nc.dram_tensor("At", [P, ZT * N_STATE], f32, kind="ExternalInput")
    dskip = nc.dram_tensor("dskip", [P, ZT], f32, kind="ExternalInput")
    woutT = nc.dram_tensor("woutT", [P, KT, ZT, P], f16, kind="ExternalInput")
    out = nc.dram_tensor("out", [DM, L], f32, kind="ExternalOutput")

    bcscr = nc.dram_tensor("bcscr", [2 * N_STATE, L], f16)   # internal
    ident_dr = nc.inline_tensor(np.eye(P, dtype=np.float16), "ident")

    with tile.TileContext(nc) as tc:
        with tc.tile_pool(name="res", bufs=1) as res, \
             tc.tile_pool(name="wpool", bufs=4) as wpool, \
             tc.tile_pool(name="bcp", bufs=3) as bcp, \
             tc.tile_pool(name="wk", bufs=2) as wk, \
             tc.tile_pool(name="scw", bufs=2) as scw, \
             tc.tile_pool(name="ps", bufs=4, space="PSUM") as ps:

            # ---- resident tiles ----
            xT_sb = res.tile([P, KT, L], f16)       # x^T, k-tile major
            xi = res.tile([P, XT, 3 + L], f16)      # pre-conv xi (3 halo cols)
            xc = res.tile([P, XT, L], f16)          # silu(conv(xi)) = u
            sz = res.tile([P, ZT, L], f16)          # silu(z)
            dt = res.tile([P, ZT, L], f16)          # softplus dt
            y2 = res.tile([P, ZT, L], f16)          # gated scan output
            xdbl = res.tile([P, L], f16)            # x_dbl rows (96 used)
            ident = res.tile([P, P], f16)
            At_sb = res.tile([P, ZT * N_STATE], f32)
            bdt_sb = res.tile([P, ZT], f32)
            dsk_sb = res.tile([P, ZT], f32)
            bcv_sb = res.tile([P, XT], f32)
            wdt_sb = res.tile([DT_RANK, DH], f16)

            nc.sync.dma_start(ident[:], ident_dr[:])
            nc.sync.dma_start(At_sb[:], At[:])
            nc.sync.dma_start(bdt_sb[:], bdt[:])
            nc.sync.dma_start(dsk_sb[:], dskip[:])
            nc.sync.dma_start(bcv_sb[:], bconv[:])
            nc.sync.dma_start(wdt_sb[:], wdtT[:])
            for k in range(KT):
                nc.sync.dma_start(xT_sb[:, k, :], xT[k * P:(k + 1) * P, :])
            for i in range(XT):
                nc.gpsimd.memset(xi[:, i, 0:3], 0.0)

            # ---- Phase A: in_proj + conv + silu ----
            for e in range(XT + ZT):                # xi tiles, then z tiles
                pacc = ps.tile([P, L], f32, tag="mm")
                wcol = wpool.tile([P, KT, P], f16, tag="wcol")
                nc.sync.dma_start(wcol[:], winT[:, e, :, :])
                for k in range(KT):
                    for f in range(NF):
                        nc.tensor.matmul(
                            pacc[:, f * FD:(f + 1) * FD], wcol[:, k, :],
                            xT_sb[:, k, f * FD:(f + 1) * FD],
                            start=(k == 0), stop=(k == KT - 1))
                if e < XT:
                    nc.scalar.copy(xi[:, e, 3:3 + L], pacc[:])
                else:
                    nc.scalar.activation(sz[:, e - XT, :], pacc[:], ACT.Silu)

            for i in range(XT):
                pcv = ps.tile([P, L], f32, tag="mm")
                cdall = wpool.tile([P, K_CONV, P], f16, tag="cd")
                nc.sync.dma_start(cdall[:], convd[:, i, :, :])
                for j in range(K_CONV):
                    for f in range(NF):
                        nc.tensor.matmul(
                            pcv[:, f * FD:(f + 1) * FD], cdall[:, j, :],
                            xi[:, i, j + f * FD: j + (f + 1) * FD],
                            start=(j == 0), stop=(j == K_CONV - 1))
                nc.scalar.activation(xc[:, i, :], pcv[:], ACT.Silu,
                                     bias=bcv_sb[:, i:i + 1])

            # ---- Phase B: x_dbl, dt, B/C rows to DRAM ----
            pxd = ps.tile([P, L], f32, tag="mm")
            for i in range(XT):
                wchunk = wpool.tile([P, NX], f16, tag="wx")
                nc.sync.dma_start(wchunk[:], wxT[i * P:(i + 1) * P, :])
                for f in range(NF):
                    nc.tensor.matmul(
                        pxd[:NX, f * FD:(f + 1) * FD], wchunk[:],
                        xc[:, i, f * FD:(f + 1) * FD],
                        start=(i == 0), stop=(i == XT - 1))
            nc.scalar.copy(xdbl[:NX, :], pxd[:NX, :])
            nc.sync.dma_start(bcscr[:], xdbl[DT_RANK:DT_RANK + 2 * N_STATE, :])

            for d in range(ZT):
                pdt = ps.tile([P, L], f32, tag="mm")
                for f in range(NF):
                    nc.tensor.matmul(
                        pdt[:, f * FD:(f + 1) * FD],
                        wdt_sb[:, d * P:(d + 1) * P],
                        xdbl[:DT_RANK, f * FD:(f + 1) * FD],
                        start=True, stop=True)
                tmp = wk.tile([P, L], f32, tag="f32tmp")
                nc.scalar.activation(tmp[:], pdt[:], ACT.Exp,
                                     bias=bdt_sb[:, d:d + 1])
                nc.scalar.activation(dt[:, d, :], tmp[:], ACT.Ln, bias=1.0)

            # ---- Phase C: selective scan, d-tile pairs ----
            for dp in range(ZT // 2):
                ds = (2 * dp, 2 * dp + 1)
                yps = {}
                dus = {}
                for d in ds:
                    ypt = ps.tile([P, L], f32, tag="mm")
                    yps[d] = ypt
                    du = wk.tile([P, L], f16, tag="du")
                    nc.vector.tensor_tensor(du[:], dt[:, d, :], xc[:, d, :],
                                            MULT)
                    dus[d] = du
                for n in range(N_STATE):
                    Bn = bcp.tile([P, L], f16, tag="Bn")
                    Cn = bcp.tile([P, L], f16, tag="Cn")
                    nc.sync.dma_start(Bn[:], bcscr[n, :].partition_broadcast(P))
                    nc.sync.dma_start(
                        Cn[:], bcscr[N_STATE + n, :].partition_broadcast(P))
                    for d in ds:
                        dA = scw.tile([P, L], f16, tag="dA")
                        nc.scalar.activation(
                            dA[:], dt[:, d, :], ACT.Exp,
                            scale=At_sb[:, d * N_STATE + n:d * N_STATE + n + 1])
                        dBu = scw.tile([P, L], f16, tag="dBu")
                        nc.vector.tensor_tensor(dBu[:], dus[d][:], Bn[:], MULT)
                        H = scw.tile([P, L], f16, tag="H")
                        nc.vector.tensor_tensor_scan(H[:], dA[:], dBu[:], 0.0,
                                                     MULT, ADD)
                        G = scw.tile([P, L], f16, tag="G")
                        nc.vector.tensor_tensor(G[:], H[:], Cn[:], MULT)
                        for f in range(NF):
                            nc.tensor.matmul(
                                yps[d][:, f * FD:(f + 1) * FD], ident[:],
                                G[:, f * FD:(f + 1) * FD],
                                start=(n == 0), stop=(n == N_STATE - 1))
                for d in ds:
                    y1 = wk.tile([P, L], f32, tag="f32tmp")
                    nc.vector.scalar_tensor_tensor(
                        y1[:], xc[:, d, :], dsk_sb[:, d:d + 1], yps[d][:],
                        MULT, ADD)
                    nc.vector.tensor_tensor(y2[:, d, :], y1[:], sz[:, d, :],
                                            MULT)

            # ---- Phase D: out_proj partial ----
            for m in range(KT):
                po = ps.tile([P, L], f32, tag="mm")
                wcol = wpool.tile([P, ZT, P], f16, tag="wcol")
                nc.sync.dma_start(wcol[:], woutT[:, m, :, :])
                for k in range(ZT):
                    for f in range(NF):
                        nc.tensor.matmul(
                            po[:, f * FD:(f + 1) * FD], wcol[:, k, :],
                            y2[:, k, f * FD:(f + 1) * FD],
                            start=(k == 0), stop=(k == ZT - 1))
                osb = wk.tile([P, L], f32, tag="f32tmp")
                nc.scalar.copy(osb[:], po[:])
                nc.sync.dma_start(out[m * P:(m + 1) * P, :], osb[:])

    nc.compile()
    return nc


def _prep_core(inputs, b, rev, half, L=1024, DM=1024, DH=1024):
    """Host-side slicing/permutation for one core's in_map.

    Channel permutation puts the core's own d_inner half at channels
    0..DH-1 so the SPMD program can use fixed tile indices for u/scan.
    """
    sfx = "r" if rev else "f"
    DI = 2 * DH
    x = np.asarray(inputs["x"])[b].astype(np.float32)     # [L, DM]
    if rev:
        x = x[::-1]
    Win = np.asarray(inputs[f"Win_{sfx}"])
    Wconv = np.asarray(inputs[f"Wconv_{sfx}"])
    bconv = np.asarray(inputs[f"bconv_{sfx}"])
    Wx = np.asarray(inputs[f"Wx_{sfx}"])
    Wdt = np.asarray(inputs[f"Wdt_{sfx}"])
    bdt = np.asarray(inputs[f"bdt_{sfx}"])
    Alog = np.asarray(inputs[f"Alog_{sfx}"])
    Dskip = np.asarray(inputs[f"Dskip_{sfx}"])
    Wout = np.asarray(inputs[f"Wout_{sfx}"])

    own = np.arange(half * DH, (half + 1) * DH)
    oth = np.arange((1 - half) * DH, (2 - half) * DH)
    perm = np.concatenate([own, oth])                     # xi channel order
    XT, ZT = DI // P, DH // P

    winT = np.concatenate(
        [Win[:DI][perm].T, Win[DI + half * DH:DI + (half + 1) * DH].T], axis=1)
    ET = (DI + DH) // P
    KT = DM // P
    winT = winT.reshape(KT, P, ET, P).transpose(1, 2, 0, 3)  # [p, e, k, c]
    Wcp = Wconv[perm].astype(np.float16)
    convd = np.zeros((P, XT, K_CONV, P), np.float16)
    pi = np.arange(P)
    for i in range(XT):
        for j in range(K_CONV):
            convd[pi, i, j, pi] = Wcp[i * P + pi, j]
    A = -np.exp(Alog[own])                                # [DH, 16]
    return {
        "xT": np.ascontiguousarray(x.T).astype(np.float16),
        "winT": np.ascontiguousarray(winT).astype(np.float16),
        "convd": convd,
        "bconv": np.ascontiguousarray(
            bconv[perm].reshape(XT, P).T).astype(np.float32),
        "wxT": np.ascontiguousarray(Wx[:, perm].T).astype(np.float16),
        "wdtT": np.ascontiguousarray(Wdt[own].T).astype(np.float16),
        "bdt": np.ascontiguousarray(
            bdt[own].reshape(ZT, P).T).astype(np.float32),
        "At": np.ascontiguousarray(
            A.reshape(ZT, P, N_STATE).transpose(1, 0, 2).reshape(
                P, ZT * N_STATE)).astype(np.float32),
        "dskip": np.ascontiguousarray(
            Dskip[own].reshape(ZT, P).T).astype(np.float32),
        "woutT": np.ascontiguousarray(Wout[:, own].T.reshape(DH // P, P, DM // P, P).transpose(1, 2, 0, 3)).astype(np.float16),
    }


_NC_CACHE = {}


def kernel(**inputs) -> np.ndarray:
    L, DM = 1024, 1024
    if "nc" not in _NC_CACHE:
        _NC_CACHE["nc"] = build(L=L, DM=DM, DH=1024)
    nc = _NC_CACHE["nc"]

    in_maps = [
        _prep_core(inputs, c // 4, bool((c // 2) % 2), c % 2)
        for c in range(8)
    ]

    import jax
    jax.devices()
    trace = os.environ.get("BIMAMBA_TRACE") == "1"
    if trace:
        from trn_agent_boot.trn_boot import _ntff_profile_via_ctypes
        import antenv.axon_hooks as ah
        if ah.get_axon_ntff_profile_hook() is None:
            ah.set_axon_ntff_profile_hook(
                _ntff_profile_via_ctypes("/opt/axon/libaxon_pjrt.so"))
    res = run_bass_kernel_spmd(nc, in_maps, list(range(8)), trace=trace)
    _NC_CACHE["exec_time_ns"] = res.exec_time_ns

    B = np.asarray(inputs["x"]).shape[0]
    outp = np.zeros((B, L, DM), np.float32)
    for c in range(8):
        b, rev = c // 4, (c // 2) % 2
        part = np.asarray(res.results[c]["out"]).T        # [L, DM]
        if rev:
            part = part[::-1]
        outp[b] += part
    return outp


# revision 8
# speedup vs baseline: 1.0362x; 1.0253x over previous
"""BiMamba (bidirectional Mamba block) on 8 TRN2 NeuronCores.

Sharding: 4 independent (batch, direction) units x 2-way split of
d_inner (2048 -> 2x1024). Core c = (b=c//4, dir=(c//2)%2, half=c%2).
All cores run ONE SPMD program; per-core differences are folded into the
host-prepared inputs (x transposed/reversed, weights sliced and channel-
permuted so the core's own d_inner half is always channels 0..1023).
Each core computes a full-depth partial of out[b] over its half; the
host sums partials, un-reverses the reverse direction, adds directions.

Per-core pipeline:
  A: in_proj (PE fp16): xi full 2048ch (x_dbl needs all) + z half;
     causal dw-conv = 4 accumulating diagonal matmuls (PE); Silu (ACT)
  B: x_dbl = Wx @ xc (PE); dt = softplus via Exp(.+bdt)+Ln(1+.) (ACT);
     B/C rows replicated to 128 partitions via 0-stride DMA
  C (per d-tile pair, per n): dA = Exp(dt*A[d,n]) (ACT per-partition
     scale); dBu = (dt*u) o B_n (DVE TT fp16 2x); h = tensor_tensor_scan
     (DVE); G = h o C_n (DVE); y = sum_n G_n via identity-matmul PSUM
     accumulation (PE); gating y2 = (y + u*Dskip)*silu(z) (DVE)
  D: out_proj partial (PE) -> DRAM fp32
"""
import os
import sys
import types

sys.path.insert(0, "/opt/trn_rl_repo")

import numpy as np

# ---- NTFF profile hook shim (trace path only; harmless otherwise) ----
if "antenv.axon_hooks" not in sys.modules:
    _m = types.ModuleType("antenv.axon_hooks")
    _m._HOOK = None
    _m.set_axon_ntff_profile_hook = lambda h, _m=_m: setattr(_m, "_HOOK", h)
    _m.get_axon_ntff_profile_hook = lambda _m=_m: _m._HOOK
    sys.modules["antenv.axon_hooks"] = _m

import concourse.bacc as bacc
import concourse.tile as tile
from concourse import mybir
from concourse.bass_utils import run_bass_kernel_spmd

f32 = mybir.dt.float32
f16 = mybir.dt.float16

DT_RANK = 64
N_STATE = 16
K_CONV = 4
P = 128


def build(L=1024, DM=1024, DH=1024):
    MULT = mybir.AluOpType.mult
    ADD = mybir.AluOpType.add
    ACT = mybir.ActivationFunctionType

    nc = bacc.Bacc("TRN2")
    DI = 2 * DH                      # full d_inner
    KT = DM // P                     # k-tiles over d_model (8)
    XT = DI // P                     # xi tiles (16)
    ZT = DH // P                     # z / scan tiles (8)
    FD = 512                         # matmul free-dim (one PSUM bank fp32)
    NF = L // FD
    NX = DT_RANK + 2 * N_STATE       # 96

    xT = nc.dram_tensor("xT", [DM, L], f16, kind="ExternalInput")
    winT = nc.dram_tensor("winT", [P, (DI + DH) // P, KT, P], f16, kind="ExternalInput")
    convd = nc.dram_tensor("convd", [P, XT, K_CONV, P], f16, kind="ExternalInput")
    bconv = nc.dram_tensor("bconv", [P, XT], f32, kind="ExternalInput")
    wxT = nc.dram_tensor("wxT", [DI, NX], f16, kind="ExternalInput")
    wdtT = nc.dram_tensor("wdtT", [DT_RANK, DH], f16, kind="ExternalInput")
    bdt = nc.dram_tensor("bdt", [P, ZT], f32, kind="ExternalInput")
    At = nc.dram_tensor("At", [P, ZT * N_STATE], f32, kind="ExternalInput")
    dskip = nc.dram_tensor("dskip", [P, ZT], f32, kind="ExternalInput")
    woutT = nc.dram_tensor("woutT", [P, KT, ZT, P], f16, kind="ExternalInput")
    out = nc.dram_tensor("out", [DM, L], f32, kind="ExternalOutput")

    bcscr = nc.dram_tensor("bcscr", [2 * N_STATE, L], f16)   # internal
    ident_dr = nc.inline_tensor(np.eye(P, dtype=np.float16), "ident")

    with tile.TileContext(nc) as tc:
        with tc.tile_pool(name="res", bufs=1) as res, \
             tc.tile_pool(name="wpool", bufs=4) as wpool, \
             tc.tile_pool(name="bcp", bufs=3) as bcp, \
             tc.tile_pool(name="wk", bufs=2) as wk, \
             tc.tile_pool(name="scw", bufs=3) as scw, \
             tc.tile_pool(name="ps", bufs=4, space="PSUM") as ps:

            # ---- resident tiles ----
            xT_sb = res.tile([P, KT, L], f16)       # x^T, k-tile major
            xi = res.tile([P, XT, 3 + L], f16)      # pre-conv xi (3 halo cols)
            xc = res.tile([P, XT, L], f16)          # silu(conv(xi)) = u
            sz = res.tile([P, ZT, L], f16)          # silu(z)
            dt = res.tile([P, ZT, L], f16)          # softplus dt
            y2 = res.tile([P, ZT, L], f16)          # gated scan output
            xdbl = res.tile([P, L], f16)            # x_dbl rows (96 used)
            ident = res.tile([P, P], f16)
            At_sb = res.tile([P, ZT * N_STATE], f32)
            bdt_sb = res.tile([P, ZT], f32)
            dsk_sb = res.tile([P, ZT], f32)
            bcv_sb = res.tile([P, XT], f32)
            wdt_sb = res.tile([DT_RANK, DH], f16)

            nc.sync.dma_start(ident[:], ident_dr[:])
            nc.sync.dma_start(At_sb[:], At[:])
            nc.sync.dma_start(bdt_sb[:], bdt[:])
            nc.sync.dma_start(dsk_sb[:], dskip[:])
            nc.sync.dma_start(bcv_sb[:], bconv[:])
            nc.sync.dma_start(wdt_sb[:], wdtT[:])
            for k in range(KT):
                nc.sync.dma_start(xT_sb[:, k, :], xT[k * P:(k + 1) * P, :])
            for i in range(XT):
                nc.gpsimd.memset(xi[:, i, 0:3], 0.0)

            # ---- Phase A: in_proj (xi tiles only; z deferred past dt) ----
            for e in range(XT):
                pacc = ps.tile([P, L], f32, tag="mm")
                wcol = wpool.tile([P, KT, P], f16, tag="wcol")
                nc.sync.dma_start(wcol[:], winT[:, e, :, :])
                for k in range(KT):
                    for f in range(NF):
                        nc.tensor.matmul(
                            pacc[:, f * FD:(f + 1) * FD], wcol[:, k, :],
                            xT_sb[:, k, f * FD:(f + 1) * FD],
                            start=(k == 0), stop=(k == KT - 1))
                nc.scalar.copy(xi[:, e, 3:3 + L], pacc[:])

            for i in range(XT):
                pcv = ps.tile([P, L], f32, tag="mm")
                cdall = wpool.tile([P, K_CONV, P], f16, tag="cd")
                nc.sync.dma_start(cdall[:], convd[:, i, :, :])
                for j in range(K_CONV):
                    for f in range(NF):
                        nc.tensor.matmul(
                            pcv[:, f * FD:(f + 1) * FD], cdall[:, j, :],
                            xi[:, i, j + f * FD: j + (f + 1) * FD],
                            start=(j == 0), stop=(j == K_CONV - 1))
                nc.scalar.activation(xc[:, i, :], pcv[:], ACT.Silu,
                                     bias=bcv_sb[:, i:i + 1])

            # ---- Phase B: x_dbl, dt, B/C rows to DRAM ----
            pxd = ps.tile([P, L], f32, tag="mm")
            for i in range(XT):
                wchunk = wpool.tile([P, NX], f16, tag="wx")
                nc.sync.dma_start(wchunk[:], wxT[i * P:(i + 1) * P, :])
                for f in range(NF):
                    nc.tensor.matmul(
                        pxd[:NX, f * FD:(f + 1) * FD], wchunk[:],
                        xc[:, i, f * FD:(f + 1) * FD],
                        start=(i == 0), stop=(i == XT - 1))
            nc.scalar.copy(xdbl[:NX, :], pxd[:NX, :])
            nc.sync.dma_start(bcscr[:], xdbl[DT_RANK:DT_RANK + 2 * N_STATE, :])

            for d in range(ZT):
                pdt = ps.tile([P, L], f32, tag="mm")
                for f in range(NF):
                    nc.tensor.matmul(
                        pdt[:, f * FD:(f + 1) * FD],
                        wdt_sb[:, d * P:(d + 1) * P],
                        xdbl[:DT_RANK, f * FD:(f + 1) * FD],
                        start=True, stop=True)
                tmp = wk.tile([P, L], f32, tag="f32tmp")
                nc.scalar.activation(tmp[:], pdt[:], ACT.Exp,
                                     bias=bdt_sb[:, d:d + 1])
                nc.scalar.activation(dt[:, d, :], tmp[:], ACT.Ln, bias=1.0)

            # z projection (feeds gating, first needed ~60us into phase C)
            for zi in range(ZT):
                pacc = ps.tile([P, L], f32, tag="mm")
                wcol = wpool.tile([P, KT, P], f16, tag="wcol")
                nc.sync.dma_start(wcol[:], winT[:, XT + zi, :, :])
                for k in range(KT):
                    for f in range(NF):
                        nc.tensor.matmul(
                            pacc[:, f * FD:(f + 1) * FD], wcol[:, k, :],
                            xT_sb[:, k, f * FD:(f + 1) * FD],
                            start=(k == 0), stop=(k == KT - 1))
                nc.scalar.activation(sz[:, zi, :], pacc[:], ACT.Silu)

            # ---- Phase C: selective scan, d-tile pairs ----
            for dp in range(ZT // 2):
                ds = (2 * dp, 2 * dp + 1)
                yps = {}
                dus = {}
                for d in ds:
                    ypt = ps.tile([P, L], f32, tag="mm")
                    yps[d] = ypt
                    du = wk.tile([P, L], f16, tag="du")
                    nc.vector.tensor_tensor(du[:], dt[:, d, :], xc[:, d, :],
                                            MULT)
                    dus[d] = du
                for n in range(N_STATE):
                    Bn = bcp.tile([P, L], f16, tag="Bn")
                    Cn = bcp.tile([P, L], f16, tag="Cn")
                    nc.sync.dma_start(Bn[:], bcscr[n, :].partition_broadcast(P))
                    nc.sync.dma_start(
                        Cn[:], bcscr[N_STATE + n, :].partition_broadcast(P))
                    for d in ds:
                        dA = scw.tile([P, L], f16, tag="dA")
                        nc.scalar.activation(
                            dA[:], dt[:, d, :], ACT.Exp,
                            scale=At_sb[:, d * N_STATE + n:d * N_STATE + n + 1])
                        dBu = scw.tile([P, L], f16, tag="dBu")
                        nc.vector.tensor_tensor(dBu[:], dus[d][:], Bn[:], MULT)
                        H = scw.tile([P, L], f16, tag="H")
                        nc.vector.tensor_tensor_scan(H[:], dA[:], dBu[:], 0.0,
                                                     MULT, ADD)
                        G = scw.tile([P, L], f16, tag="G")
                        nc.vector.tensor_tensor(G[:], H[:], Cn[:], MULT)
                        for f in range(NF):
                            nc.tensor.matmul(
                                yps[d][:, f * FD:(f + 1) * FD], ident[:],
                                G[:, f * FD:(f + 1) * FD],
                                start=(n == 0), stop=(n == N_STATE - 1))
                for d in ds:
                    y1 = wk.tile([P, L], f32, tag="f32tmp")
                    nc.vector.scalar_tensor_tensor(
                        y1[:], xc[:, d, :], dsk_sb[:, d:d + 1], yps[d][:],
                        MULT, ADD)
                    nc.vector.tensor_tensor(y2[:, d, :], y1[:], sz[:, d, :],
                                            MULT)

            # ---- Phase D: out_proj partial ----
            for m in range(KT):
                po = ps.tile([P, L], f32, tag="mm")
                wcol = wpool.tile([P, ZT, P], f16, tag="wcol")
                nc.sync.dma_start(wcol[:], woutT[:, m, :, :])
                for k in range(ZT):
                    for f in range(NF):
                        nc.tensor.matmul(
                            po[:, f * FD:(f + 1) * FD], wcol[:, k, :],
                            y2[:, k, f * FD:(f + 1) * FD],
                            start=(k == 0), stop=(k == ZT - 1))
                osb = wk.tile([P, L], f32, tag="f32tmp")
                nc.scalar.copy(osb[:], po[:])
                nc.sync.dma_start(out[m * P:(m + 1) * P, :], osb[:])

    nc.compile()
    return nc


def _prep_core(inputs, b, rev, half, L=1024, DM=1024, DH=1024):
    """Host-side slicing/permutation for one core's in_map.

    Channel permutation puts the core's own d_inner half at channels
    0..DH-1 so the SPMD program can use fixed tile indices for u/scan.
    """
    sfx = "r" if rev else "f"
    DI = 2 * DH
    x = np.asarray(inputs["x"])[b].astype(np.float32)     # [L, DM]
    if rev:
        x = x[::-1]
    Win = np.asarray(inputs[f"Win_{sfx}"])
    Wconv = np.asarray(inputs[f"Wconv_{sfx}"])
    bconv = np.asarray(inputs[f"bconv_{sfx}"])
    Wx = np.asarray(inputs[f"Wx_{sfx}"])
    Wdt = np.asarray(inputs[f"Wdt_{sfx}"])
    bdt = np.asarray(inputs[f"bdt_{sfx}"])
    Alog = np.asarray(inputs[f"Alog_{sfx}"])
    Dskip = np.asarray(inputs[f"Dskip_{sfx}"])
    Wout = np.asarray(inputs[f"Wout_{sfx}"])

    own = np.arange(half * DH, (half + 1) * DH)
    oth = np.arange((1 - half) * DH, (2 - half) * DH)
    perm = np.concatenate([own, oth])                     # xi channel order
    XT, ZT = DI // P, DH // P

    winT = np.concatenate(
        [Win[:DI][perm].T, Win[DI + half * DH:DI + (half + 1) * DH].T], axis=1)
    ET = (DI + DH) // P
    KT = DM // P
    winT = winT.reshape(KT, P, ET, P).transpose(1, 2, 0, 3)  # [p, e, k, c]
    Wcp = Wconv[perm].astype(np.float16)
    convd = np.zeros((P, XT, K_CONV, P), np.float16)
    pi = np.arange(P)
    for i in range(XT):
        for j in range(K_CONV):
            convd[pi, i, j, pi] = Wcp[i * P + pi, j]
    A = -np.exp(Alog[own])                                # [DH, 16]
    return {
        "xT": np.ascontiguousarray(x.T).astype(np.float16),
        "winT": np.ascontiguousarray(winT).astype(np.float16),
        "convd": convd,
        "bconv": np.ascontiguousarray(
            bconv[perm].reshape(XT, P).T).astype(np.float32),
        "wxT": np.ascontiguousarray(Wx[:, perm].T).astype(np.float16),
        "wdtT": np.ascontiguousarray(Wdt[own].T).astype(np.float16),
        "bdt": np.ascontiguousarray(
            bdt[own].reshape(ZT, P).T).astype(np.float32),
        "At": np.ascontiguousarray(
            A.reshape(ZT, P, N_STATE).transpose(1, 0, 2).reshape(
                P, ZT * N_STATE)).astype(np.float32),
        "dskip": np.ascontiguousarray(
            Dskip[own].reshape(ZT, P).T).astype(np.float32),
        "woutT": np.ascontiguousarray(Wout[:, own].T.reshape(DH // P, P, DM // P, P).transpose(1, 2, 0, 3)).astype(np.float16),
    }


_NC_CACHE = {}


def kernel(**inputs) -> np.ndarray:
    L, DM = 1024, 1024
    if "nc" not in _NC_CACHE:
        _NC_CACHE["nc"] = build(L=L, DM=DM, DH=1024)
    nc = _NC_CACHE["nc"]

    in_maps = [
        _prep_core(inputs, c // 4, bool((c // 2) % 2), c % 2)
        for c in range(8)
    ]

    import jax
    jax.devices()
    trace = os.environ.get("BIMAMBA_TRACE") == "1"
    if trace:
        from trn_agent_boot.trn_boot import _ntff_profile_via_ctypes
        import antenv.axon_hooks as ah
        if ah.get_axon_ntff_profile_hook() is None:
            ah.set_axon_ntff_profile_hook(
                _ntff_profile_via_ctypes("/opt/axon/libaxon_pjrt.so"))
    res = run_bass_kernel_spmd(nc, in_maps, list(range(8)), trace=trace)
    _NC_CACHE["exec_time_ns"] = res.exec_time_ns

    B = np.asarray(inputs["x"]).shape[0]
    outp = np.zeros((B, L, DM), np.float32)
    for c in range(8):
        b, rev = c // 4, (c // 2) % 2
        part = np.asarray(res.results[c]["out"]).T        # [L, DM]
        if rev:
            part = part[::-1]
        outp[b] += part
    return outp
